# revision 27
# baseline (speedup 1.0000x reference)
"""Banded dense-dilated KNN graph (k=9, band 90, dilation 1) on 8 Trainium2 cores.

Input  x: (4, 64, 8192, 1) float32.
Output e: (2, 4, 8192, 9) int32 = stack([nn_idx, center_idx]).

Algorithm notes
---------------
The reference L2-normalizes x over the 64-dim feature axis and takes, per row
i, the 9 smallest banded distances d(i,j) = 2 - 2 u_i.u_j for j in [i-89, i].
Rank 0 is always j == i (self), and the within-row ordering of the remaining
candidates is the ordering of the dot products u_i.u_j descending.  The host
pre-normalizes u = x/|x| (fp64) and ships it as bf16; the device computes, per
64-row sub-block, the [64 x 153] window of dot products and extracts the
per-row top-8 with the DVE max8/max_index instructions.  Two independent
64-row sub-blocks (one from each 2048-row half of the core's range) are packed
into each 128-partition tile: their feature vectors live on disjoint partition
quadrants (0-63 / 64-127), so two K=64 matmuls with partition-offset PSUM
outputs fill one [128 x 153] tile.  The band mask (0 / -1e30) is accumulated
into PSUM first via an identity-stationary matmul, so the masked scores come
out of the accumulation directly; one ACT copy moves them to SBUF fp32 for the
DVE scan.  Self (rank 0), the first-8-row head fixup, and the center-index
plane are reconstructed on the host, which is exact.

Sharding: 8 cores = 4 batches x 2 row-halves of 4096 rows; no cross-core
communication.  Each core gets its own 4096 rows plus the 89 preceding
columns (zero padding for the batch-leading half), stacked as two 64-feature
halves on the partition quadrants.
"""

import sys

import numpy as np

for _p in ("/opt/trn_rl_repo", "/root/.axon_site/_ro/trn_rl_repo"):
    if _p not in sys.path:
        sys.path.append(_p)

B = 4
D = 64
N = 8192
K = 9
LB = 90  # band width (j in [i-89, i])
W = LB - 1  # 89 back-columns
HALF = N // 2  # rows per core
QROW = HALF // 2  # 2048 rows per stacked half
SUB = 32  # rows per sub-block (4 sub-blocks per 128-partition tile)
WID = SUB + W  # 121-column window per sub-block
NT = QROW // 64  # 32 tiles per core (each covers 64 rows of both halves)
HCOLS = W + QROW  # 2137 columns per stacked half
BIG = 1.0e30

_CACHED = {}

# input-DMA waves: cumulative U columns unlock 4 / 17 / 32 tiles
# (tile t needs cols [64t, 64t+153); 345 = 64*3+153, 1241 = 64*17+153)
MEGA = [(0, 345), (345, 896), (1241, 896)]


def _subchunks(c0, cw, step=512):
    out = []
    o = c0
    while o < c0 + cw:
        w = min(step, c0 + cw - o)
        out.append((o, w))
        o += w
    return out


def _build_masks():
    """[128, WID] bf16 masks: 0 where column c is a valid neighbor of the
    sub-block row r = p % 32, -1e30 otherwise.  Valid (non-self) neighbors of
    global row i = r0 + r are j in [i-89, i-1] -> c = j - (r0 - 89) in
    [r, r+88].  Partition group g = p // 32 of tile t holds sub-block rows
    starting at 64t + 32*(g%2) (halves on g//2); batch-leading halves
    additionally require j >= 0, i.e. c >= 89 - (64t + 32*(g%2)) for the
    first two tiles."""
    import ml_dtypes

    r = np.arange(SUB)[:, None]
    c = np.arange(WID)[None, :]
    valid = (c >= r) & (c <= r + W - 1)

    def mk(cmin_g0, cmin_g1):
        g0 = np.where(valid & (c >= cmin_g0), 0.0, -BIG).astype(np.float32)
        g1 = np.where(valid & (c >= cmin_g1), 0.0, -BIG).astype(np.float32)
        rest = np.where(valid, 0.0, -BIG).astype(np.float32)
        return np.vstack([g0, g1, rest, rest]).astype(ml_dtypes.bfloat16)

    return mk(89, 57), mk(25, 0), mk(0, 0)


def _build_bass():
    import concourse.mybir as mybir
    from concourse import bacc
    from concourse.tile import TileContext

    f32 = mybir.dt.float32
    bf16 = mybir.dt.bfloat16
    u16 = mybir.dt.uint16
    Act = mybir.ActivationFunctionType

    nc = bacc.Bacc("TRN2", target_bir_lowering=False, debug=False, num_devices=8)
    # one input tensor: [identity(128) | m0 | m1 | mr (121 each) | U(2137)];
    # the first DMA grabs consts + U wave 0 in a single transfer so tile 0's
    # whole working set arrives with one DMA latency.
    CW = 128 + 3 * WID  # 491 const columns
    us_d = nc.dram_tensor("us", [128, CW + HCOLS], bf16, kind="ExternalInput")
    idx_d = nc.dram_tensor("idx_out", [128, NT * 8], u16, kind="ExternalOutput")

    with TileContext(nc) as tc:
        with (
            tc.tile_pool(name="big", bufs=1) as big,
            tc.tile_pool(name="consts", bufs=1) as consts,
            tc.tile_pool(name="psd", bufs=6, space="PSUM") as psd,
            tc.tile_pool(name="psw", bufs=1, space="PSUM") as psw,
            tc.tile_pool(name="sco", bufs=6) as sco,
            tc.tile_pool(name="out8", bufs=6) as out8,
        ):
            UF = big.tile([128, CW + HCOLS], bf16, tag="UF")
            ID = UF[:, 0:128]
            m0 = UF[:, 128 : 128 + WID]
            m1 = UF[:, 128 + WID : 128 + 2 * WID]
            mr = UF[:, 128 + 2 * WID : 128 + 3 * WID]
            IDX = big.tile([128, NT * 8], u16, tag="IDX")

            # first DMA: consts + U wave 0 in ONE transfer on the Pool SWDGE
            # queue — its 25ns SEQ + desc-gen beats the HWDGE queue's
            # 565+632+650ns launch stack, so tile 0's working set lands ~0.7us
            # sooner.
            nc.gpsimd.dma_start(UF[:, 0 : CW + 345], us_d[:, 0 : CW + 345])

            # Warm the ACT Copy function table immediately so the ~1.3us
            # table load overlaps the input DMAs.
            warm = consts.tile([2, 2], f32, tag="warm")
            nc.vector.memset(warm[:], 1.0)
            nc.scalar.activation(warm[:], warm[:], Act.Copy)

            # PE p-state pre-ramp: dummy matmuls on scratch data while the
            # input DMAs are in flight, so the first real tiles run at full
            # PE speed instead of the 2x mid-pstate penalty.
            scr = consts.tile([128, 512], bf16, tag="scr")
            nc.vector.memset(scr[:], 0.0)
            pwarm = psw.tile([128, 512], f32, tag="pwarm")
            for _ in range(5):
                nc.tensor.matmul(
                    pwarm[:], lhsT=scr[:, 0:128], rhs=scr[:], start=True, stop=True
                )

            def load_wave(mi):
                if mi == 0:
                    return  # wave 0 rode the first DMA
                c0, cw = MEGA[mi]
                for si, (s0, sw) in enumerate(_subchunks(c0, cw)):
                    eng = nc.sync if si % 2 == 0 else nc.scalar
                    eng.dma_start(
                        UF[:, CW + s0 : CW + s0 + sw],
                        us_d[:, CW + s0 : CW + s0 + sw],
                    )

            def tile_block(t):
                m = m0 if t == 0 else (m1 if t == 1 else mr)
                pd = psd.tile([128, WID], f32, tag="pd")
                nc.tensor.matmul(pd[:], lhsT=ID, rhs=m, start=True, stop=False)
                for g in range(4):
                    q, sub = g // 2, g % 2
                    base = CW + 64 * t + SUB * sub
                    nc.tensor.matmul(
                        pd[32 * g : 32 * g + 32, :],
                        lhsT=UF[64 * q : 64 * q + 64, W + base : W + base + SUB],
                        rhs=UF[64 * q : 64 * q + 64, base : base + WID],
                        start=False,
                        stop=True,
                        tile_position=(64 * q, 32 * g),
                    )
                sc = sco.tile([128, WID], f32, tag="sc")
                nc.scalar.activation(sc[:], pd[:], Act.Copy)
                vals = out8.tile([128, 8], f32, tag="vals")
                nc.vector.max(out=vals[:], in_=sc[:])
                nc.vector.max_index(
                    out=IDX[:, 8 * t : 8 * (t + 1)], in_max=vals[:], in_values=sc[:]
                )

            # Wave-pipelined emission: each DMA wave is followed by the tiles
            # it unlocks; later waves overlap earlier tile work.
            unlocked = [4, 18, NT]
            emitted = 0
            for mi in range(len(MEGA)):
                load_wave(mi)
                while emitted < unlocked[mi]:
                    tile_block(emitted)
                    emitted += 1
                    if emitted == 16:
                        nc.sync.dma_start(idx_d[:, :128], IDX[:, :128])
                    elif emitted == 28:
                        # ship tiles 16-27 while 28-31 compute: the final
                        # store after the last max_index is then tiny.
                        nc.sync.dma_start(idx_d[:, 128:224], IDX[:, 128:224])
            # final store on the idle Pool SWDGE queue: 25ns SEQ + desc-gen
            # beats the HWDGE launch stack on the critical tail.
            nc.gpsimd.dma_start(idx_d[:, 224:], IDX[:, 224:])

    nc.finalize()
    return nc


LAST_EXEC_NS = None


def kernel(x: np.ndarray) -> np.ndarray:
    global LAST_EXEC_NS
    import os

    import ml_dtypes
    from concourse import bass_utils

    if "nc" not in _CACHED:
        _CACHED["nc"] = _build_bass()
        _CACHED["masks"] = _build_masks()
    nc = _CACHED["nc"]
    m_first0, m_first1, m_rest = _CACHED["masks"]

    x = np.asarray(x)
    assert x.shape == (B, D, N, 1) and x.dtype == np.float32
    xm = x[:, :, :, 0].astype(np.float64)  # (B, D, N)
    norms = np.sqrt((xm * xm).sum(axis=1, keepdims=True))
    u = (xm / np.maximum(norms, 1e-12)).astype(np.float32).astype(ml_dtypes.bfloat16)

    ident = np.eye(128, dtype=np.float32).astype(ml_dtypes.bfloat16)
    cst_lead = np.hstack([ident, m_first0, m_first1, m_rest])
    cst_rest = np.hstack([ident, m_rest, m_rest, m_rest])
    CW = cst_lead.shape[1]

    in_maps = []
    for core in range(8):
        b, h = core // 2, core % 2
        us = np.zeros((128, CW + HCOLS), ml_dtypes.bfloat16)
        # batch-leading half: tiles 0/1 need boundary masks
        us[:, :CW] = cst_lead if h == 0 else cst_rest
        for q in range(2):
            half_start = h * HALF + q * QROW
            lo = half_start - W
            src0 = max(lo, 0)
            us[64 * q : 64 * q + 64, CW + (src0 - lo) :] = u[
                b, :, src0 : half_start + QROW
            ]
        in_maps.append({"us": us})

    trace = os.environ.get("KNN_TRACE", "0") == "1"
    res = bass_utils.run_bass_kernel_spmd(
        nc, in_maps, core_ids=list(range(8)), trace=trace
    )
    LAST_EXEC_NS = res.exec_time_ns

    # --- host-side unshard + index reconstruction (exact) ---
    nn = np.empty((B, N, K), np.int64)
    rows = np.arange(HALF)
    offs = rows // SUB * SUB - W  # window base col per local row
    for core in range(8):
        b, h = core // 2, core % 2
        start = h * HALF
        raw = res.results[core]["idx_out"].astype(np.int64)  # [128, NT*8]
        # partition p = 32*(2q + sub) + r, col = t*8 + k
        #   -> local row q*2048 + 64t + 32*sub + r
        c = (
            raw.reshape(2, 2, 32, NT, 8)
            .transpose(0, 3, 1, 2, 4)
            .reshape(HALF, 8)
        )
        nn[b, start : start + HALF, 1:] = c + (start + offs)[:, None]
    nn[:, :, 0] = np.arange(N)[None, :]
    # Head fixup: row i < 8 has only i valid non-self neighbors; reference
    # fills columns k > i with the self index.
    for i in range(K - 1):
        nn[:, i, i + 1 :] = i
    center = np.broadcast_to(np.arange(N)[None, :, None], (B, N, K))
    return np.stack([nn, center], axis=0).astype(np.int32)


# revision 32
# speedup vs baseline: 1.0438x; 1.0438x over previous
"""Banded dense-dilated KNN graph (k=9, band 90, dilation 1) on 8 Trainium2 cores.

Input  x: (4, 64, 8192, 1) float32.
Output e: (2, 4, 8192, 9) int32 = stack([nn_idx, center_idx]).

Algorithm notes
---------------
The reference L2-normalizes x over the 64-dim feature axis and takes, per row
i, the 9 smallest banded distances d(i,j) = 2 - 2 u_i.u_j for j in [i-89, i].
Rank 0 is always j == i (self), and the within-row ordering of the remaining
candidates is the ordering of the dot products u_i.u_j descending.  The host
pre-normalizes u = x/|x| (fp64) and ships it as bf16; the device computes, per
64-row sub-block, the [64 x 153] window of dot products and extracts the
per-row top-8 with the DVE max8/max_index instructions.  Two independent
64-row sub-blocks (one from each 2048-row half of the core's range) are packed
into each 128-partition tile: their feature vectors live on disjoint partition
quadrants (0-63 / 64-127), so two K=64 matmuls with partition-offset PSUM
outputs fill one [128 x 153] tile.  The band mask (0 / -1e30) is accumulated
into PSUM first via an identity-stationary matmul, so the masked scores come
out of the accumulation directly; one ACT copy moves them to SBUF fp32 for the
DVE scan.  Self (rank 0), the first-8-row head fixup, and the center-index
plane are reconstructed on the host, which is exact.

Sharding: 8 cores = 4 batches x 2 row-halves of 4096 rows; no cross-core
communication.  Each core gets its own 4096 rows plus the 89 preceding
columns (zero padding for the batch-leading half), stacked as two 64-feature
halves on the partition quadrants.
"""

import sys

import numpy as np

for _p in ("/opt/trn_rl_repo", "/root/.axon_site/_ro/trn_rl_repo"):
    if _p not in sys.path:
        sys.path.append(_p)

B = 4
D = 64
N = 8192
K = 9
LB = 90  # band width (j in [i-89, i])
W = LB - 1  # 89 back-columns
HALF = N // 2  # rows per core
QROW = HALF // 2  # 2048 rows per stacked half
SUB = 32  # rows per sub-block (4 sub-blocks per 128-partition tile)
WID = SUB + W  # 121-column window per sub-block
NT = QROW // 64  # 32 tiles per core (each covers 64 rows of both halves)
HCOLS = W + QROW  # 2137 columns per stacked half
BIG = 1.0e30

_CACHED = {}

# input-DMA waves: cumulative U columns unlock 4 / 17 / 32 tiles
# (tile t needs cols [64t, 64t+153); 345 = 64*3+153, 1241 = 64*17+153)
MEGA = [(0, 345), (345, 896), (1241, 896)]


def _subchunks(c0, cw, step=512):
    out = []
    o = c0
    while o < c0 + cw:
        w = min(step, c0 + cw - o)
        out.append((o, w))
        o += w
    return out


def _build_masks():
    """[128, WID] bf16 masks: 0 where column c is a valid neighbor of the
    sub-block row r = p % 32, -1e30 otherwise.  Valid (non-self) neighbors of
    global row i = r0 + r are j in [i-89, i-1] -> c = j - (r0 - 89) in
    [r, r+88].  Partition group g = p // 32 of tile t holds sub-block rows
    starting at 64t + 32*(g%2) (halves on g//2); batch-leading halves
    additionally require j >= 0, i.e. c >= 89 - (64t + 32*(g%2)) for the
    first two tiles."""
    import ml_dtypes

    r = np.arange(SUB)[:, None]
    c = np.arange(WID)[None, :]
    valid = (c >= r) & (c <= r + W - 1)

    def mk(cmin_g0, cmin_g1):
        g0 = np.where(valid & (c >= cmin_g0), 0.0, -BIG).astype(np.float32)
        g1 = np.where(valid & (c >= cmin_g1), 0.0, -BIG).astype(np.float32)
        rest = np.where(valid, 0.0, -BIG).astype(np.float32)
        return np.vstack([g0, g1, rest, rest]).astype(ml_dtypes.bfloat16)

    return mk(89, 57), mk(25, 0), mk(0, 0)


def _build_bass():
    import concourse.mybir as mybir
    from concourse import bacc
    from concourse.tile import TileContext

    f32 = mybir.dt.float32
    bf16 = mybir.dt.bfloat16
    u16 = mybir.dt.uint16
    Act = mybir.ActivationFunctionType

    nc = bacc.Bacc("TRN2", target_bir_lowering=False, debug=False, num_devices=8)
    # one input tensor: [identity(128) | m0 | m1 (121 each) | U(2137) | mr];
    # the first DMA grabs [ID|m0|m1|U[0:217]] in a single transfer so the
    # working set of tiles 0-1 arrives with one DMA latency (mr is only
    # needed from tile 2 and rides the second transfer).
    CW = 128 + 2 * WID  # 370 leading const columns
    us_d = nc.dram_tensor(
        "us", [128, CW + HCOLS + WID], bf16, kind="ExternalInput"
    )
    idx_d = nc.dram_tensor("idx_out", [128, NT * 8], u16, kind="ExternalOutput")

    with TileContext(nc) as tc:
        with (
            tc.tile_pool(name="big", bufs=1) as big,
            tc.tile_pool(name="consts", bufs=1) as consts,
            tc.tile_pool(name="psd", bufs=6, space="PSUM") as psd,
            tc.tile_pool(name="psw", bufs=1, space="PSUM") as psw,
            tc.tile_pool(name="sco", bufs=6) as sco,
            tc.tile_pool(name="out8", bufs=6) as out8,
        ):
            UF = big.tile([128, CW + HCOLS + WID], bf16, tag="UF")
            ID = UF[:, 0:128]
            m0 = UF[:, 128 : 128 + WID]
            m1 = UF[:, 128 + WID : 128 + 2 * WID]
            mr = UF[:, CW + HCOLS : CW + HCOLS + WID]
            IDX = big.tile([128, NT * 8], u16, tag="IDX")

            # first DMA on the SP queue: consts + U[0:217] in ONE transfer —
            # the whole working set of tiles 0-1 with one DMA latency.
            nc.sync.dma_start(UF[:, 0 : CW + 217], us_d[:, 0 : CW + 217])
            # second transfer (other queue): U[217:466] + trailing mr, as two
            # DMAs so the mr mask (needed by tile 2) isn't gated on bulk U.
            nc.scalar.dma_start(
                UF[:, CW + HCOLS :], us_d[:, CW + HCOLS :]
            )
            nc.sync.dma_start(
                UF[:, CW + 217 : CW + 466], us_d[:, CW + 217 : CW + 466]
            )

            # Warm the ACT Copy function table immediately so the ~1.3us
            # table load overlaps the input DMAs.
            warm = consts.tile([2, 2], f32, tag="warm")
            nc.vector.memset(warm[:], 1.0)
            nc.scalar.activation(warm[:], warm[:], Act.Copy)

            # PE p-state pre-ramp: dummy matmuls on scratch data while the
            # input DMAs are in flight, so the first real tiles run at full
            # PE speed instead of the 2x mid-pstate penalty.
            scr = consts.tile([128, 512], bf16, tag="scr")
            nc.gpsimd.memset(scr[:], 0.0)
            pwarm = psw.tile([128, 512], f32, tag="pwarm")
            for _ in range(5):
                nc.tensor.matmul(
                    pwarm[:], lhsT=scr[:, 0:128], rhs=scr[:], start=True, stop=True
                )

            def load_wave(mi):
                if mi == 0:
                    return  # wave 0 rode the first DMA
                c0, cw = MEGA[mi]
                for si, (s0, sw) in enumerate(_subchunks(c0, cw)):
                    eng = nc.sync if si % 2 == 0 else nc.scalar
                    eng.dma_start(
                        UF[:, CW + s0 : CW + s0 + sw],
                        us_d[:, CW + s0 : CW + s0 + sw],
                    )

            def tile_block(t):
                m = m0 if t == 0 else (m1 if t == 1 else mr)
                pd = psd.tile([128, WID], f32, tag="pd")
                nc.tensor.matmul(pd[:], lhsT=ID, rhs=m, start=True, stop=False)
                for g in range(4):
                    q, sub = g // 2, g % 2
                    base = CW + 64 * t + SUB * sub
                    nc.tensor.matmul(
                        pd[32 * g : 32 * g + 32, :],
                        lhsT=UF[64 * q : 64 * q + 64, W + base : W + base + SUB],
                        rhs=UF[64 * q : 64 * q + 64, base : base + WID],
                        start=False,
                        stop=True,
                        tile_position=(64 * q, 32 * g),
                    )
                sc = sco.tile([128, WID], f32, tag="sc")
                nc.scalar.activation(sc[:], pd[:], Act.Copy)
                vals = out8.tile([128, 8], f32, tag="vals")
                nc.vector.max(out=vals[:], in_=sc[:])
                nc.vector.max_index(
                    out=IDX[:, 8 * t : 8 * (t + 1)], in_max=vals[:], in_values=sc[:]
                )

            # Wave-pipelined emission: each DMA wave is followed by the tiles
            # it unlocks; later waves overlap earlier tile work.
            unlocked = [4, 18, NT]
            emitted = 0
            for mi in range(len(MEGA)):
                load_wave(mi)
                while emitted < unlocked[mi]:
                    tile_block(emitted)
                    emitted += 1
                    if emitted == 16:
                        nc.sync.dma_start(idx_d[:, :128], IDX[:, :128])
                    elif emitted == 28:
                        # ship tiles 16-27 while 28-31 compute: the final
                        # store after the last max_index is then tiny.
                        nc.sync.dma_start(idx_d[:, 128:224], IDX[:, 128:224])
            nc.scalar.dma_start(idx_d[:, 224:], IDX[:, 224:])

    nc.finalize()
    return nc


LAST_EXEC_NS = None


def kernel(x: np.ndarray) -> np.ndarray:
    global LAST_EXEC_NS
    import os

    import ml_dtypes
    from concourse import bass_utils

    if "nc" not in _CACHED:
        _CACHED["nc"] = _build_bass()
        _CACHED["masks"] = _build_masks()
    nc = _CACHED["nc"]
    m_first0, m_first1, m_rest = _CACHED["masks"]

    x = np.asarray(x)
    assert x.shape == (B, D, N, 1) and x.dtype == np.float32
    xm = x[:, :, :, 0].astype(np.float64)  # (B, D, N)
    norms = np.sqrt((xm * xm).sum(axis=1, keepdims=True))
    u = (xm / np.maximum(norms, 1e-12)).astype(np.float32).astype(ml_dtypes.bfloat16)

    ident = np.eye(128, dtype=np.float32).astype(ml_dtypes.bfloat16)
    cst_lead = np.hstack([ident, m_first0, m_first1, m_rest])
    cst_rest = np.hstack([ident, m_rest, m_rest, m_rest])
    CW = cst_lead.shape[1]

    in_maps = []
    for core in range(8):
        b, h = core // 2, core % 2
        us = np.zeros((128, CW + HCOLS), ml_dtypes.bfloat16)
        # batch-leading half: tiles 0/1 need boundary masks
        us[:, :CW] = cst_lead if h == 0 else cst_rest
        for q in range(2):
            half_start = h * HALF + q * QROW
            lo = half_start - W
            src0 = max(lo, 0)
            us[64 * q : 64 * q + 64, CW + (src0 - lo) :] = u[
                b, :, src0 : half_start + QROW
            ]
        in_maps.append({"us": us})

    trace = os.environ.get("KNN_TRACE", "0") == "1"
    res = bass_utils.run_bass_kernel_spmd(
        nc, in_maps, core_ids=list(range(8)), trace=trace
    )
    LAST_EXEC_NS = res.exec_time_ns

    # --- host-side unshard + index reconstruction (exact) ---
    nn = np.empty((B, N, K), np.int64)
    rows = np.arange(HALF)
    offs = rows // SUB * SUB - W  # window base col per local row
    for core in range(8):
        b, h = core // 2, core % 2
        start = h * HALF
        raw = res.results[core]["idx_out"].astype(np.int64)  # [128, NT*8]
        # partition p = 32*(2q + sub) + r, col = t*8 + k
        #   -> local row q*2048 + 64t + 32*sub + r
        c = (
            raw.reshape(2, 2, 32, NT, 8)
            .transpose(0, 3, 1, 2, 4)
            .reshape(HALF, 8)
        )
        nn[b, start : start + HALF, 1:] = c + (start + offs)[:, None]
    nn[:, :, 0] = np.arange(N)[None, :]
    # Head fixup: row i < 8 has only i valid non-self neighbors; reference
    # fills columns k > i with the self index.
    for i in range(K - 1):
        nn[:, i, i + 1 :] = i
    center = np.broadcast_to(np.arange(N)[None, :, None], (B, N, K))
    return np.stack([nn, center], axis=0).astype(np.int32)
